# revision 1
# baseline (speedup 1.0000x reference)
"""Trainium2 Bass kernel for the sum-product "knowledge layer" network.

Computation (see problem reference):
  h0 = encode(x): 8194-row table [-inf, 0, pos0, neg0, pos1, neg1, ...]
       with pos = x (log-probs), neg = log(1 - exp(x)), per batch column.
  4 alternating layers, each: gather rows by ptrs, then segment-reduce over
  contiguous fanin groups (fanin 4 sum-of-logs "product" layers, fanin 2
  logsumexp "sum" layers).

Strategy (pure batch data-parallelism, 8 NeuronCores):
  - Shard the 512 batch columns 8 ways -> 64 columns per core.
  - Per core every tensor lives in DRAM as [rows, 64] fp32; one row = 256B.
  - Gathers use the SWDGE dma_gather instruction: int16 index list in SBUF,
    each index pulls one 256B row from the DRAM table; index list position j
    lands at SBUF partition j%128, free slot j//128.
  - DAG pruning (host side, per input set): working back from the 2048
    output rows, only rows actually referenced downstream are computed.
  - Layer fusion: layers 0+1 fuse into stage A, layers 2+3 into stage B.
    A stage group gathers its 8 leaf rows (2 product groups x fanin 4),
    sums each quad on DVE, then logsumexps the pair - so the intermediate
    product table never exists in DRAM, removing its store and a pipeline
    boundary, at the cost of recomputing product rows referenced by more
    than one sum edge (~4% more gather descriptors).
  - Cross-layer software pipelining: stage A's output rows are produced in
    chunk order (chunk ci stores rows [base, base + P*csub), row = base +
    p*csub + cc), A groups are sorted by the max table-0 row they
    reference, and every gather chunk's source AP is narrowed to the exact
    table prefix it needs, so the tile framework only serializes a gather
    against the stores that overlap its prefix. The encode is likewise
    chunked (vars are laid out slot-major: var v lives at partition v%128,
    slot v//128, so encode chunk j fills a table-0 row prefix). Chunk
    sizes ramp up then down so desc-gen and compute tails stay short.
  - Sum reduction: lse(a,b) = ln(e^a + e^b) computed directly (1 DVE add;
    Exp/Ln on the lightly-loaded ACT engine) - resolve_direct() verifies
    on the host, from the actual x values, that every e^arg stays a
    normal f32, falling back to max + ln(1+exp(min-max)) otherwise.  The
    Exp+Ln activation table is preloaded once (set 6) so the compiler
    inserts no per-instruction table reloads.
"""

import numpy as np

P = 128
B = 64  # batch columns per core
NCORES = 8
N_VARS = 4096
BATCH = 512
TAB0 = 2 * N_VARS + 2  # 8194
OUT_SIZES = [16384, 8192, 4096, 2048]
FANINS = [4, 2, 4, 2]
FE = 8  # edges per fused group: 2 (sum fanin) x 4 (product fanin)
CHUNK = 9216  # max gather indices per dma_gather instruction
S_ENC = N_VARS // P  # 32 encode slots per partition
ENC_CHUNKS = 4
SE = S_ENC // ENC_CHUNKS  # slots per encode chunk


def _pad_groups(n):
    return -(-n // P) * P


def _chunk_group_counts(n_groups, tail_ramp=(128, 128, 128, 256, 512)):
    """Groups per dma_gather chunk (FE edges per group). Sizes ramp up at
    the start (small first chunk -> quick desc-gen once the source prefix
    lands) and down at the end (short compute tail -> the last store lands
    early and the consumer unblocks sooner)."""
    g = P
    rem = n_groups
    tail = []
    for s in tail_ramp:
        if rem >= s + g:
            tail.append(s)
            rem -= s
    head = []
    for s in (128, 256, 512):
        if rem >= s + g:
            head.append(s)
            rem -= s
    mid = []
    per = CHUNK // FE
    while rem > 0:
        s = min(per, rem)
        mid.append(s)
        rem -= s
    return head + mid + tail[::-1]


def plan(ptrs_list):
    """Prune the DAG bottom-up, fuse layer pairs, readiness-order stage A.

    Returns (stageA, stageB) dicts:
      n_groups: padded group count (stage A: pruned sum-layer-1 groups =
                rows of the intermediate table tA; stage B: 2048 outputs)
      n_src_rows: rows of the gathered table (A: TAB0, B: nA)
      chunks: list of (n_groups_in_chunk, src_prefix_rows)
      edge_src: per-edge source rows, production order, FE per group
    """
    p0, p1, p2, p3 = [np.asarray(p).astype(np.int64) for p in ptrs_list]
    # stage B: out group g needs L2 groups p3[2g], p3[2g+1]; each L2 group
    # h needs t-A rows p2[4h+k].
    b_l2 = p3.reshape(-1, 2)  # [2048, 2] L2 group ids
    b_src1 = p2.reshape(-1, 4)[b_l2]  # [2048, 2, 4] L1 (tA) compact... raw ids
    used1 = np.unique(b_src1)
    # stage A: one group per used L1 row; L1 row u needs L0 groups
    # p1[2u+j]; L0 group w needs t0 rows p0[4w+k].
    a_l0 = p1.reshape(-1, 2)[used1]  # [n1, 2] L0 group ids
    a_src0 = p0.reshape(-1, 4)[a_l0]  # [n1, 2, 4] t0 rows

    n1 = used1.size
    nA = _pad_groups(n1)
    srcA = np.zeros((nA, FE), dtype=np.int64)
    srcA[:n1] = a_src0.reshape(n1, FE)
    # readiness of a t0 row: encode chunk order (var slot v//128; rows 0/1
    # ready first)
    ready = np.maximum(srcA - 2, 0) // 2 // P
    rmax = ready.max(axis=1)
    # padding groups read only row 0: ready before any encode chunk, so
    # putting them first lets chunk 0's desc-gen+gather warm up under the
    # encode instead of idling the DMA engines.
    rmax[n1:] = -1
    order = np.argsort(rmax, kind="stable")
    srcA = srcA[order]
    prod = np.empty(nA, dtype=np.int64)
    prod[order] = np.arange(nA)  # compact A-group id -> production row

    relabel1 = prod[np.searchsorted(used1, b_src1)]  # [2048, 2, 4] tA rows
    srcB = relabel1.reshape(-1, FE)
    # Stage B output rows need no fixed order either - the host unpermutes
    # rows after the run - so readiness-sort B too: its early chunks then
    # only need a tA prefix and can overlap stage A's tail.
    orderB = np.argsort(srcB.max(axis=1), kind="stable")
    srcB = srcB[orderB]
    prodB = np.empty(srcB.shape[0], dtype=np.int64)
    prodB[orderB] = np.arange(srcB.shape[0])  # out group g -> produced row

    def mk(src, n_src_rows, tail_ramp):
        n_groups = src.shape[0]
        chunks = []
        g_off = 0
        for gc in _chunk_group_counts(n_groups, tail_ramp):
            m = int(src[g_off : g_off + gc].max()) + 1
            chunks.append((gc, m))
            g_off += gc
        return {
            "n_groups": n_groups,
            "n_src_rows": n_src_rows,
            "chunks": chunks,
            "edge_src": src.ravel(),
        }

    # stage A gets an extra-fine tail: its last tA stores land earlier,
    # unblocking stage B's high-prefix chunks sooner.
    stages = [
        mk(srcA, TAB0, (128, 128, 128, 128, 128, 256)),
        mk(srcB, nA, (128, 128, 128, 256, 512)),
    ]
    stages[1]["out_perm"] = prodB

    # Logsumexp form per stage: lse(a,b) = ln(e^a + e^b) directly costs
    # 1 DVE op (vs 4) + whole-tile Exp on the lightly-loaded ACT engine,
    # but is only safe when e^min stays a normal f32.  resolve_direct()
    # checks the exact values once x is known.
    stages[0]["direct"] = None
    stages[1]["direct"] = None
    stages[0]["_srcB_quads"] = srcB.reshape(-1, 2, 4)
    return stages


def resolve_direct(stages, x):
    """Exact host-side bound check for the stage-B direct-form lse."""
    x = np.asarray(x, dtype=np.float64)
    pos = x
    neg = np.log1p(-np.exp(x))
    t0 = np.empty((TAB0, x.shape[1]))
    t0[0] = 0.0
    t0[1] = 0.0
    t0[2::2] = pos
    t0[3::2] = neg
    srcA = stages[0]["edge_src"].reshape(-1, 2, 4)
    q = t0[srcA].sum(axis=2)  # [nA, 2, cols]
    m = q.max(axis=1)
    mn = q.min(axis=1)
    tA = m + np.log1p(np.exp(mn - m))
    tb = tA[stages[0]["_srcB_quads"]].sum(axis=2)  # [2048, 2, cols]
    safe_a = q.min() > -80.0
    safe_b = tb.min() > -80.0
    stages[0]["direct"] = bool(safe_a)
    stages[1]["direct"] = bool(safe_b)


def reorder_wrap(stage):
    """Permute per-edge source ids into dma_gather order and wrap into the
    int16 [128, n_edges//16] SBUF layout (position j -> [j%16, j//16],
    replicated across the 8 gpsimd cores' 16-partition groups).

    Edge position j of chunk ci maps to partition p=j%128, slot=j//128,
    cc=slot//FE, k=slot%FE, production row = base_ci + p*csub + cc."""
    src = stage["edge_src"]
    out = np.empty(stage["n_groups"] * FE, dtype=np.int64)
    base = 0
    e_off = 0
    for gc, _m in stage["chunks"]:
        csub = gc // P
        n_e = gc * FE
        j = np.arange(n_e)
        p = j % P
        slot = j // P
        cc = slot // FE
        k = slot % FE
        row = base + p * csub + cc
        out[e_off : e_off + n_e] = src[row * FE + k]
        base += gc
        e_off += n_e
    assert out.max() < 2**15 and out.min() >= 0
    out = out.astype(np.int16)
    return np.ascontiguousarray(np.tile(out.reshape(-1, 16).T, (8, 1)))


def build_nc(meta):
    """meta: per-stage (n_groups, n_src_rows, chunks-tuple)."""
    import concourse.bacc as bacc
    import concourse.mybir as mybir
    import concourse.tile as tile

    f32 = mybir.dt.float32
    i16 = mybir.dt.int16
    Alu = mybir.AluOpType
    Act = mybir.ActivationFunctionType

    specs = [
        {"n_groups": n, "n_src_rows": s, "chunks": ch, "direct": dr}
        for (n, s, ch, dr) in meta
    ]

    nc = bacc.Bacc("TRN2", target_bir_lowering=False, debug=False)
    x = nc.dram_tensor("x", [P, S_ENC * B], f32, kind="ExternalInput")
    idx_in = [
        nc.dram_tensor(
            f"idx{l}", [P, FE * s["n_groups"] // 16], i16, kind="ExternalInput"
        )
        for l, s in enumerate(specs)
    ]
    out = nc.dram_tensor("out", [OUT_SIZES[3], B], f32, kind="ExternalOutput")

    with tile.TileContext(nc) as tc:
        with (
            tc.tile_pool(name="dram", bufs=1, space="DRAM") as dpool,
            tc.tile_pool(name="sb", bufs=4) as gp,
            tc.tile_pool(name="enc", bufs=ENC_CHUNKS) as ep,
            tc.tile_pool(name="hb", bufs=4) as hp,
            tc.tile_pool(name="tmp", bufs=3) as tp,
            tc.tile_pool(name="ix", bufs=1) as ixp,
        ):
            tables = [
                dpool.tile([s["n_src_rows"], B], f32, name=f"t{l}", tag=f"t{l}")
                for l, s in enumerate(specs)
            ]

            # Preload the combined Exp+Ln activation table once; the
            # insert_act_table_loads pass then finds every Exp/Ln already
            # covered and inserts no per-instruction reloads (1283ns each).
            ACT_SET_LN_EXP = 6  # natural_log_exp_and_others
            nc.scalar.add_instruction(
                mybir.InstLoadActFuncSet(
                    name=nc.get_next_instruction_name(),
                    ins=[],
                    outs=[],
                    act_func_set_id=ACT_SET_LN_EXP,
                )
            )

            # table0 rows 0 (-inf in the reference, never gathered) and 1
            # (zeros). Store first so the row prefix [0,2) is ready.
            z = ixp.tile([2, B], f32, tag="z")
            nc.vector.memset(z[:], 0.0)
            nc.sync.dma_start(tables[0][:][0:2, :], z[:])

            # --- encode, chunked: var v sits at partition v%128, slot
            # v//128; pos row 2+2v, neg row 3+2v.  Chunk j covers slots
            # [j*SE,(j+1)*SE) = rows [2+256*j*SE, 2+256*(j+1)*SE), a row
            # prefix, so stage-A gather chunks can start before the whole
            # encode finishes.
            # All independent loads are emitted before any store so the
            # in-order DMA queue never has a compute-gated store blocking a
            # ready load: x chunks first, then the index lists.
            xv = x[:].rearrange("p (s b) -> p s b", b=B)
            xls = []
            for j in range(ENC_CHUNKS):
                # contiguous destination: 2KB DMA descriptors (the
                # interleaved iv[:, :, 0, :] view would halve them to 256B
                # and pay the sub-512B 2x transfer penalty)
                xl = ep.tile([P, SE, B], f32, tag="xl")
                nc.sync.dma_start(xl[:], xv[:, j * SE : (j + 1) * SE, :])
                xls.append(xl)
            ix_t = []
            for l, s in enumerate(specs):
                t = ixp.tile([P, FE * s["n_groups"] // 16], i16, tag=f"ix{l}")
                nc.sync.dma_start(t[:], idx_in[l][:])
                ix_t.append(t)
            for j in range(ENC_CHUNKS):
                xl = xls[j]
                iv = ep.tile([P, SE, 2, B], f32, tag="enc")
                et = hp.tile([P, SE, B], f32, tag="h")
                nc.scalar.copy(iv[:][:, :, 0, :], xl[:])
                nc.scalar.activation(et[:], xl[:], Act.Exp)
                nc.scalar.activation(
                    iv[:][:, :, 1, :], et[:], Act.Ln, scale=-1.0, bias=1.0
                )
                r0 = 2 + 2 * P * SE * j
                r1 = 2 + 2 * P * SE * (j + 1)
                # row = r0 + 256*s + 2*p + k
                nc.sync.dma_start(
                    tables[0][:][r0:r1, :].rearrange("(s p k) b -> p s k b", p=P, k=2),
                    iv[:],
                )

            # --- fused gather + product-sum + logsumexp stages ---
            for l, s in enumerate(specs):
                dst_tile = tables[l + 1][:] if l + 1 < len(specs) else out[:]
                g_off = 0
                e_off = 0
                for gc, m_src in s["chunks"]:
                    csub = gc // P
                    ch = gc * FE
                    S = ch // P
                    g = gp.tile([P, S, B], f32, tag="g")
                    nc.gpsimd.dma_gather(
                        g[:],
                        tables[l][:][0:m_src, :],
                        ix_t[l][:, e_off // 16 : (e_off + ch) // 16],
                        ch,
                        ch,
                        B,
                        single_packet=False,
                    )
                    # [p, group, pair(2), fanin(4), b]
                    v = g[:].rearrange("p (c j k) b -> p c j k b", j=2, k=4)
                    s01 = tp.tile([P, csub, 2, B], f32, tag="m")
                    s23 = tp.tile([P, csub, 2, B], f32, tag="n")
                    ss = gp.tile([P, csub, 2, B], f32, tag="s")
                    nc.vector.tensor_add(s01[:], v[:, :, :, 0, :], v[:, :, :, 1, :])
                    nc.vector.tensor_add(s23[:], v[:, :, :, 2, :], v[:, :, :, 3, :])
                    nc.vector.tensor_add(ss[:], s01[:], s23[:])
                    a = ss[:][:, :, 0, :]
                    b = ss[:][:, :, 1, :]
                    h = hp.tile([P, csub, B], f32, tag="h")
                    if s["direct"]:
                        # lse(a,b) = ln(e^a + e^b): host verified e^min is a
                        # normal f32 (no scaling needed). 1 DVE op; the
                        # whole-tile Exp and the Ln ride the ACT engine.
                        e = tp.tile([P, csub, 2, B], f32, tag="d")
                        d = tp.tile([P, csub, B], f32, tag="sp")
                        nc.scalar.activation(e[:], ss[:], Act.Exp)
                        nc.vector.tensor_add(
                            d[:], e[:][:, :, 0, :], e[:][:, :, 1, :]
                        )
                        nc.scalar.activation(h[:], d[:], Act.Ln)
                    else:
                        # wider range: logsumexp = max + ln(1+exp(min-max))
                        m = tp.tile([P, csub, B], f32, tag="d")
                        mn = tp.tile([P, csub, B], f32, tag="sp")
                        sp = tp.tile([P, csub, B], f32, tag="sq")
                        nc.vector.tensor_tensor(m[:], a, b, op=Alu.max)
                        nc.vector.tensor_tensor(mn[:], a, b, op=Alu.min)
                        nc.vector.tensor_tensor(mn[:], mn[:], m[:], op=Alu.subtract)
                        nc.scalar.activation(mn[:], mn[:], Act.Exp)
                        nc.scalar.activation(sp[:], mn[:], Act.Ln, bias=1.0)
                        nc.vector.tensor_add(h[:], m[:], sp[:])
                    # chunk produces rows [g_off, g_off + P*csub):
                    # row = g_off + p*csub + cc
                    nc.sync.dma_start(
                        dst_tile[g_off : g_off + P * csub, :].rearrange(
                            "(p c) b -> p (c b)", p=P
                        ),
                        h[:].rearrange("p c b -> p (c b)"),
                    )
                    g_off += P * csub
                    e_off += ch
    nc.compile()
    return nc


def host_prep(x, ptrs_list, seg_list):
    """Host-side sharding + pruning + index preprocessing -> per-core maps."""
    x = np.asarray(x, dtype=np.float32)
    for l, (n_out, f) in enumerate(zip(OUT_SIZES, FANINS)):
        seg = np.asarray(seg_list[l]).astype(np.int64)
        expected = np.repeat(np.arange(n_out, dtype=np.int64), f)
        assert np.array_equal(seg, expected), f"layer {l}: non-uniform segments"

    stages = plan(ptrs_list)
    idx_maps = {f"idx{l}": reorder_wrap(s) for l, s in enumerate(stages)}

    batch = x.shape[1]
    bpc = batch // NCORES
    in_maps = []
    for i in range(NCORES):
        xs = x[:, i * bpc : (i + 1) * bpc]
        # partition p, slot s holds var s*128+p (slot-major var layout)
        xv = np.ascontiguousarray(
            xs.reshape(S_ENC, P, bpc).transpose(1, 0, 2)
        ).reshape(P, -1)
        in_maps.append({"x": xv, **idx_maps})
    return in_maps


def _meta(stages):
    return tuple(
        (s["n_groups"], s["n_src_rows"], tuple(s["chunks"]), bool(s["direct"]))
        for s in stages
    )


_CACHE = {}


def _get_nc(meta=None):
    if meta is None:
        meta = _CACHE.get("meta")
        assert meta is not None, "call kernel() first"
    if _CACHE.get("meta") != meta:
        _CACHE["nc"] = build_nc(meta)
        _CACHE["meta"] = meta
    return _CACHE["nc"]


def kernel(x, ptrs0, seg0, ptrs1, seg1, ptrs2, seg2, ptrs3, seg3):
    from concourse.bass_utils import run_bass_kernel_spmd

    ptrs_list = [ptrs0, ptrs1, ptrs2, ptrs3]
    stages = plan(ptrs_list)
    resolve_direct(stages, x)
    nc = _get_nc(_meta(stages))
    in_maps = host_prep(x, ptrs_list, [seg0, seg1, seg2, seg3])
    res = run_bass_kernel_spmd(nc, in_maps, core_ids=list(range(NCORES)))
    outs = [r["out"] for r in res.results]
    full = np.concatenate(outs, axis=1)
    # rows were produced in readiness order; map back to natural order
    return np.ascontiguousarray(full[stages[1]["out_perm"]])



# revision 10
# speedup vs baseline: 1.3420x; 1.3420x over previous
"""Trainium2 Bass kernel for the sum-product "knowledge layer" network.

Computation (see problem reference):
  h0 = encode(x): 8194-row table [-inf, 0, pos0, neg0, pos1, neg1, ...]
       with pos = x (log-probs), neg = log(1 - exp(x)), per batch column.
  4 alternating layers, each: gather rows by ptrs, then segment-reduce over
  contiguous fanin groups (fanin 4 sum-of-logs "product" layers, fanin 2
  logsumexp "sum" layers).

Strategy (2-way batch x 4-way output-group sharding, 8 NeuronCores):
  - The DMA cost model charges per gather descriptor max(bytes, 512)/bus:
    sub-512B descriptors run at half rate, and the gather ISA requires
    256B-multiple rows.  64-col fp32 rows (256B) therefore cost exactly as
    much as 256-col fp16 rows (512B) PER DESCRIPTOR - so we make each
    descriptor carry 4x the batch columns at half precision and shard the
    *output groups* instead of sharding the batch further.
  - Shard the 512 batch columns 2 ways (256 per core, one fp16 table row =
    512B); within a column block, shard the 2048 output groups 4 ways.
    Each core computes only the stage-A (fused layer 0+1) groups its own
    output quarter references: ~2.6K of the 5.9K used groups (the quarters
    are chosen by sorting output groups on their min stage-A reference,
    which clusters shared references and cuts ~11% of the redundancy).
    Group computations are duplicated ~1.8x across the 4 quarters, but
    descriptor count per core drops 4x - net ~2x less DMA time, with zero
    inter-core communication.
  - Per core every table lives in DRAM as [rows, 256] fp16; one row = 512B.
  - Gathers use the SWDGE dma_gather instruction: int16 index list in SBUF,
    each index pulls one 512B row from the DRAM table.
  - DAG pruning (host side, per input set): working back from the core's
    512 output rows, only rows actually referenced downstream are computed.
  - Layer fusion: layers 0+1 fuse into stage A, layers 2+3 into stage B
    (8 gathered leaf rows per group: 2 product quads, summed on DVE, then
    logsumexp'd).
  - Cross-layer software pipelining: stage A's output rows are produced in
    chunk order, A groups are sorted by the max table-0 row they reference,
    and every gather chunk's source AP is narrowed to the exact table
    prefix it needs, so the tile framework only serializes a gather
    against the stores that overlap its prefix.  The encode is likewise
    chunked, with vars laid out partition-major per chunk (var
    1024c + 8p + s at partition p, slot s of chunk c) so each partition
    stores one contiguous 8KB run per chunk and chunk c fills the row
    prefix [2+2048c, 2+2048(c+1)).
  - All four quarters are padded to a common group count and share one
    compiled program (chunk source prefixes take the max over quarters);
    only the index lists and x differ per core.
  - Sum reduction: lse(a,b) = ln(e^a + e^b) computed directly (1 DVE add;
    Exp/Ln on the lightly-loaded ACT engine) - resolve_direct() verifies
    on the host, from the actual x values, that every e^arg stays a
    normal f32, falling back to max + ln(1+exp(min-max)) otherwise.  The
    Exp+Ln activation table is preloaded once (set 6) so the compiler
    inserts no per-instruction table reloads.
  - fp16 tables keep ample accuracy for the 2e-2 gate: host-emulated
    pipeline error is ~1.6e-3 max on the reference data.
"""

import numpy as np

P = 128
B = 256  # batch columns per core (2-way batch shard)
NCOLB = 2  # column blocks
NQ = 4  # output-group quarters per column block
NCORES = NCOLB * NQ
N_VARS = 4096
BATCH = 512
TAB0 = 2 * N_VARS + 2  # 8194
OUT_SIZES = [16384, 8192, 4096, 2048]
FANINS = [4, 2, 4, 2]
FE = 8  # edges per fused group: 2 (sum fanin) x 4 (product fanin)
GPC = 256  # max gather groups per dma_gather chunk (2048 indices)
S_ENC = N_VARS // P  # 32 encode slots per partition
ENC_CHUNKS = 4
SE = S_ENC // ENC_CHUNKS  # slots per encode chunk
OUT_PER_Q = OUT_SIZES[3] // NQ  # 512 output groups per quarter


def _pad_groups(n):
    return -(-n // P) * P


def _chunk_group_counts(n_groups, tail_ramp):
    """Groups per dma_gather chunk (FE edges per group). Sizes ramp up at
    the start (small first chunk -> quick desc-gen once the source prefix
    lands) and down at the end (short compute tail -> the last store lands
    early and the consumer unblocks sooner)."""
    g = P
    rem = n_groups
    tail = []
    for s in tail_ramp:
        if rem >= s + g:
            tail.append(s)
            rem -= s
    head = []
    for s in (128, 256):
        if rem >= s + g:
            head.append(s)
            rem -= s
    mid = []
    while rem > 0:
        s = min(GPC, rem)
        mid.append(s)
        rem -= s
    return head + mid + tail[::-1]


def plan(ptrs_list):
    """Partition output groups into quarters, prune each quarter's DAG
    bottom-up, fuse layer pairs, readiness-order stage A.

    Returns (stageA, stageB) dicts with common (padded) shape metadata and
    per-quarter index data:
      n_groups: padded group count, common across quarters
      n_src_rows: rows of the gathered table (A: TAB0, B: nA)
      chunks: list of (n_groups_in_chunk, src_prefix_rows); prefixes are
              maxed over quarters so one compiled program serves all cores
      edge_src: per-quarter list of per-edge source rows, production order
    """
    p0, p1, p2, p3 = [np.asarray(p).astype(np.int64) for p in ptrs_list]
    b_l2 = p3.reshape(-1, 2)  # [2048, 2] L2 group ids
    b_src1 = p2.reshape(-1, 4)[b_l2]  # [2048, 2, 4] L1 ids per out group
    # Quarter assignment: sort out-groups by min L1 reference so groups
    # sharing stage-A work land in the same quarter (cuts ~11% of the
    # cross-quarter duplication vs contiguous quarters).
    qorder = np.argsort(b_src1.reshape(OUT_SIZES[3], -1).min(axis=1), kind="stable")
    quarters = [qorder[OUT_PER_Q * q : OUT_PER_Q * (q + 1)] for q in range(NQ)]

    per_q = []
    for q in range(NQ):
        bq = b_src1[quarters[q]]  # [512, 2, 4]
        used1 = np.unique(bq)
        a_l0 = p1.reshape(-1, 2)[used1]  # [n1, 2] L0 group ids
        a_src0 = p0.reshape(-1, 4)[a_l0]  # [n1, 2, 4] t0 rows
        per_q.append({"used1": used1, "a_src0": a_src0, "bq": bq, "ids": quarters[q]})

    nA = max(_pad_groups(pq["used1"].size) for pq in per_q)

    for pq in per_q:
        n1 = pq["used1"].size
        srcA = np.zeros((nA, FE), dtype=np.int64)
        srcA[:n1] = pq["a_src0"].reshape(n1, FE)
        # readiness of a t0 row: encode chunk order (var v=(r-2)//2 ready
        # with encode chunk v//1024; rows 0/1 ready first)
        ready = np.maximum(srcA - 2, 0) // 2 // P
        rmax = ready.max(axis=1)
        # padding groups read only row 0: ready before any encode chunk, so
        # putting them first lets chunk 0's desc-gen+gather warm up under
        # the encode instead of idling the DMA engines.
        rmax[n1:] = -1
        order = np.argsort(rmax, kind="stable")
        pq["srcA"] = srcA[order]
        prod = np.empty(nA, dtype=np.int64)
        prod[order] = np.arange(nA)  # compact A-group id -> production row

        relabel1 = prod[np.searchsorted(pq["used1"], pq["bq"])]  # [512, 2, 4]
        srcB = relabel1.reshape(-1, FE)
        # Stage B rows need no fixed order either - the host unpermutes
        # rows after the run - so readiness-sort B too: its early chunks
        # then only need a tA prefix and can overlap stage A's tail.
        orderB = np.argsort(srcB.max(axis=1), kind="stable")
        pq["srcB"] = srcB[orderB]
        prodB = np.empty(srcB.shape[0], dtype=np.int64)
        prodB[orderB] = np.arange(srcB.shape[0])  # local out g -> produced row
        pq["out_prod"] = prodB

    def mk(srcs, n_src_rows, tail_ramp):
        n_groups = srcs[0].shape[0]
        chunks = []
        g_off = 0
        for gc in _chunk_group_counts(n_groups, tail_ramp):
            m = max(int(s[g_off : g_off + gc].max()) + 1 for s in srcs)
            chunks.append((gc, m))
            g_off += gc
        return {
            "n_groups": n_groups,
            "n_src_rows": n_src_rows,
            "chunks": chunks,
            "edge_src": [s.ravel() for s in srcs],
        }

    stages = [
        mk([pq["srcA"] for pq in per_q], TAB0, (128, 128, 128, 256)),
        mk([pq["srcB"] for pq in per_q], nA, (128, 128)),
    ]
    stages[1]["out_prod"] = [pq["out_prod"] for pq in per_q]
    stages[1]["out_ids"] = [pq["ids"] for pq in per_q]
    stages[0]["direct"] = None
    stages[1]["direct"] = None
    return stages


def resolve_direct(stages, x):
    """Exact host-side bound check for the direct-form lse (e^arg must stay
    a normal f32), evaluated over all quarters."""
    x = np.asarray(x, dtype=np.float64)
    pos = x
    neg = np.log1p(-np.exp(x))
    t0 = np.empty((TAB0, x.shape[1]))
    t0[0] = 0.0
    t0[1] = 0.0
    t0[2::2] = pos
    t0[3::2] = neg
    mn_a = 0.0
    mn_b = 0.0
    for q in range(NQ):
        srcA = stages[0]["edge_src"][q].reshape(-1, 2, 4)
        qv = t0[srcA].sum(axis=2)  # [nA, 2, cols]
        m = qv.max(axis=1)
        lo = qv.min(axis=1)
        tA = m + np.log1p(np.exp(lo - m))
        srcB = stages[1]["edge_src"][q].reshape(-1, 2, 4)
        tb = tA[srcB].sum(axis=2)
        mn_a = min(mn_a, qv.min())
        mn_b = min(mn_b, tb.min())
    stages[0]["direct"] = bool(mn_a > -80.0)
    stages[1]["direct"] = bool(mn_b > -80.0)


def reorder_wrap(stage, q):
    """Permute quarter q's per-edge source ids into dma_gather order and
    wrap into the int16 [128, n_edges//16] SBUF layout (position j ->
    [j%16, j//16], replicated across the 8 gpsimd cores' 16-partition
    groups).

    Edge position j of chunk ci maps to partition p=j%128, slot=j//128,
    cc=slot//FE, k=slot%FE, production row = base_ci + p*csub + cc."""
    src = stage["edge_src"][q]
    out = np.empty(stage["n_groups"] * FE, dtype=np.int64)
    base = 0
    e_off = 0
    for gc, _m in stage["chunks"]:
        csub = gc // P
        n_e = gc * FE
        j = np.arange(n_e)
        p = j % P
        slot = j // P
        cc = slot // FE
        k = slot % FE
        row = base + p * csub + cc
        out[e_off : e_off + n_e] = src[row * FE + k]
        base += gc
        e_off += n_e
    assert out.max() < 2**15 and out.min() >= 0
    out = out.astype(np.int16)
    return np.ascontiguousarray(np.tile(out.reshape(-1, 16).T, (8, 1)))


def build_nc(meta):
    """meta: per-stage (n_groups, n_src_rows, chunks-tuple, direct)."""
    import concourse.bacc as bacc
    import concourse.mybir as mybir
    import concourse.tile as tile

    f32 = mybir.dt.float32
    f16 = mybir.dt.float16
    i16 = mybir.dt.int16
    Alu = mybir.AluOpType
    Act = mybir.ActivationFunctionType

    specs = [
        {"n_groups": n, "n_src_rows": s, "chunks": ch, "direct": dr}
        for (n, s, ch, dr) in meta
    ]

    nc = bacc.Bacc("TRN2", target_bir_lowering=False, debug=False)
    x = nc.dram_tensor("x", [P, S_ENC * B], f16, kind="ExternalInput")
    idx_in = [
        nc.dram_tensor(
            f"idx{l}", [P, FE * s["n_groups"] // 16], i16, kind="ExternalInput"
        )
        for l, s in enumerate(specs)
    ]
    out = nc.dram_tensor("out", [OUT_PER_Q, B], f32, kind="ExternalOutput")

    with tile.TileContext(nc) as tc:
        with (
            tc.tile_pool(name="dram", bufs=1, space="DRAM") as dpool,
            tc.tile_pool(name="sb", bufs=4) as gp,
            tc.tile_pool(name="enc", bufs=ENC_CHUNKS) as ep,
            tc.tile_pool(name="hb", bufs=4) as hp,
            tc.tile_pool(name="tmp", bufs=3) as tp,
            tc.tile_pool(name="ix", bufs=1) as ixp,
        ):
            tables = [
                dpool.tile([s["n_src_rows"], B], f16, name=f"t{l}", tag=f"t{l}")
                for l, s in enumerate(specs)
            ]

            # Preload the combined Exp+Ln activation table once; the
            # insert_act_table_loads pass then finds every Exp/Ln already
            # covered and inserts no per-instruction reloads (1283ns each).
            ACT_SET_LN_EXP = 6  # natural_log_exp_and_others
            nc.scalar.add_instruction(
                mybir.InstLoadActFuncSet(
                    name=nc.get_next_instruction_name(),
                    ins=[],
                    outs=[],
                    act_func_set_id=ACT_SET_LN_EXP,
                )
            )

            # table0 rows 0 (-inf in the reference, never gathered) and 1
            # (zeros). Store first so the row prefix [0,2) is ready.
            z = ixp.tile([2, B], f16, tag="z")
            nc.vector.memset(z[:], 0.0)
            nc.sync.dma_start(tables[0][:][0:2, :], z[:])

            # --- encode, chunked: var v sits at partition (v%1024)//SE,
            # slot v%SE of chunk v//1024; pos row 2+2v, neg row 3+2v.
            # Chunk j covers rows [2+2048j, 2+2048(j+1)), a row prefix, so
            # stage-A gather chunks can start before the whole encode
            # finishes.  Within a chunk each partition's 2*SE rows are
            # contiguous, so the fp16 store is 128 runs of 8KB (full-rate
            # DMA; interleaved layouts would pay the sub-512B 2x penalty).
            # All independent loads are emitted before any store so the
            # in-order DMA queue never has a compute-gated store blocking a
            # ready load: x chunks first, then the index lists.
            xv = x[:].rearrange("p (s b) -> p s b", b=B)
            xls = []
            for j in range(ENC_CHUNKS):
                xl = ep.tile([P, SE, B], f16, tag="xl")
                nc.sync.dma_start(xl[:], xv[:, j * SE : (j + 1) * SE, :])
                xls.append(xl)
            ix_t = []
            for l, s in enumerate(specs):
                t = ixp.tile([P, FE * s["n_groups"] // 16], i16, tag=f"ix{l}")
                nc.sync.dma_start(t[:], idx_in[l][:])
                ix_t.append(t)
            for j in range(ENC_CHUNKS):
                xl = xls[j]
                iv = ep.tile([P, SE, 2, B], f16, tag="enc")
                et = hp.tile([P, SE, B], f32, tag="h")
                nc.scalar.copy(iv[:][:, :, 0, :], xl[:])
                nc.scalar.activation(et[:], xl[:], Act.Exp)
                nc.scalar.activation(
                    iv[:][:, :, 1, :], et[:], Act.Ln, scale=-1.0, bias=1.0
                )
                r0 = 2 + 2 * P * SE * j
                r1 = 2 + 2 * P * SE * (j + 1)
                # row = r0 + 2*SE*p + 2*s + k
                nc.sync.dma_start(
                    tables[0][:][r0:r1, :].rearrange("(p s k) b -> p s k b", s=SE, k=2),
                    iv[:],
                )

            # --- fused gather + product-sum + logsumexp stages ---
            for l, s in enumerate(specs):
                last = l + 1 == len(specs)
                dst_tile = out[:] if last else tables[l + 1][:]
                # intermediate tables are fp16 (halves gather descriptor
                # bytes); the final output must stay fp32
                h_dt = f32 if last else f16
                g_off = 0
                e_off = 0
                for gc, m_src in s["chunks"]:
                    csub = gc // P
                    ch = gc * FE
                    S = ch // P
                    g = gp.tile([P, S, B], f16, tag="g")
                    nc.gpsimd.dma_gather(
                        g[:],
                        tables[l][:][0:m_src, :],
                        ix_t[l][:, e_off // 16 : (e_off + ch) // 16],
                        ch,
                        ch,
                        B,
                        single_packet=False,
                    )
                    # [p, group, pair(2), fanin(4), b]
                    v = g[:].rearrange("p (c j k) b -> p c j k b", j=2, k=4)
                    s01 = tp.tile([P, csub, 2, B], f16, tag="m")
                    s23 = tp.tile([P, csub, 2, B], f16, tag="n")
                    ss = gp.tile([P, csub, 2, B], f16, tag="s")
                    nc.vector.tensor_add(s01[:], v[:, :, :, 0, :], v[:, :, :, 1, :])
                    nc.vector.tensor_add(s23[:], v[:, :, :, 2, :], v[:, :, :, 3, :])
                    nc.vector.tensor_add(ss[:], s01[:], s23[:])
                    a = ss[:][:, :, 0, :]
                    b = ss[:][:, :, 1, :]
                    h = hp.tile([P, csub, B], h_dt, tag="h")
                    if s["direct"]:
                        # lse(a,b) = ln(e^a + e^b): host verified e^min is a
                        # normal f32 (no scaling needed). 1 DVE op; the
                        # whole-tile Exp and the Ln ride the ACT engine.
                        e = tp.tile([P, csub, 2, B], f32, tag="d")
                        d = tp.tile([P, csub, B], f32, tag="sp")
                        nc.scalar.activation(e[:], ss[:], Act.Exp)
                        nc.vector.tensor_add(
                            d[:], e[:][:, :, 0, :], e[:][:, :, 1, :]
                        )
                        nc.scalar.activation(h[:], d[:], Act.Ln)
                    else:
                        # wider range: logsumexp = max + ln(1+exp(min-max))
                        m = tp.tile([P, csub, B], f32, tag="d")
                        mn = tp.tile([P, csub, B], f32, tag="sp")
                        sp = tp.tile([P, csub, B], f32, tag="sq")
                        nc.vector.tensor_tensor(m[:], a, b, op=Alu.max)
                        nc.vector.tensor_tensor(mn[:], a, b, op=Alu.min)
                        nc.vector.tensor_tensor(mn[:], mn[:], m[:], op=Alu.subtract)
                        nc.scalar.activation(mn[:], mn[:], Act.Exp)
                        nc.scalar.activation(sp[:], mn[:], Act.Ln, bias=1.0)
                        nc.vector.tensor_add(h[:], m[:], sp[:])
                    # chunk produces rows [g_off, g_off + P*csub):
                    # row = g_off + p*csub + cc
                    nc.sync.dma_start(
                        dst_tile[g_off : g_off + P * csub, :].rearrange(
                            "(p c) b -> p (c b)", p=P
                        ),
                        h[:].rearrange("p c b -> p (c b)"),
                    )
                    g_off += P * csub
                    e_off += ch
    nc.compile()
    return nc


def host_prep(x, ptrs_list, seg_list, stages=None):
    """Host-side sharding + pruning + index preprocessing -> per-core maps."""
    x = np.asarray(x, dtype=np.float32)
    for l, (n_out, f) in enumerate(zip(OUT_SIZES, FANINS)):
        seg = np.asarray(seg_list[l]).astype(np.int64)
        expected = np.repeat(np.arange(n_out, dtype=np.int64), f)
        assert np.array_equal(seg, expected), f"layer {l}: non-uniform segments"

    if stages is None:
        stages = plan(ptrs_list)
    idx_maps = [
        {f"idx{l}": reorder_wrap(s, q) for l, s in enumerate(stages)}
        for q in range(NQ)
    ]

    xvs = []
    for cb in range(NCOLB):
        xs = x[:, cb * B : (cb + 1) * B].astype(np.float16)
        # partition p, slot j*SE+s holds var 1024j + SE*p + s
        # (partition-major within each encode chunk; see build_nc)
        xv = np.ascontiguousarray(
            xs.reshape(ENC_CHUNKS, P, SE, B).transpose(1, 0, 2, 3)
        ).reshape(P, -1)
        xvs.append(xv)
    # core i -> column block i % NCOLB, quarter i // NCOLB
    return [{"x": xvs[i % NCOLB], **idx_maps[i // NCOLB]} for i in range(NCORES)]


def _meta(stages):
    return tuple(
        (s["n_groups"], s["n_src_rows"], tuple(s["chunks"]), bool(s["direct"]))
        for s in stages
    )


_CACHE = {}


def _get_nc(meta=None):
    if meta is None:
        meta = _CACHE.get("meta")
        assert meta is not None, "call kernel() first"
    if _CACHE.get("meta") != meta:
        _CACHE["nc"] = build_nc(meta)
        _CACHE["meta"] = meta
    return _CACHE["nc"]


def kernel(x, ptrs0, seg0, ptrs1, seg1, ptrs2, seg2, ptrs3, seg3):
    from concourse.bass_utils import run_bass_kernel_spmd

    ptrs_list = [ptrs0, ptrs1, ptrs2, ptrs3]
    stages = plan(ptrs_list)
    resolve_direct(stages, x)
    nc = _get_nc(_meta(stages))
    in_maps = host_prep(x, ptrs_list, [seg0, seg1, seg2, seg3], stages)
    res = run_bass_kernel_spmd(nc, in_maps, core_ids=list(range(NCORES)))
    full = np.empty((OUT_SIZES[3], BATCH), dtype=np.float32)
    for i in range(NCORES):
        cb, q = i % NCOLB, i // NCOLB
        rows = res.results[i]["out"][stages[1]["out_prod"][q]]
        full[stages[1]["out_ids"][q], cb * B : (cb + 1) * B] = rows
    return full


# revision 17
# speedup vs baseline: 1.5521x; 1.1566x over previous
"""Trainium2 Bass kernel for the sum-product "knowledge layer" network.

Computation (see problem reference):
  h0 = encode(x): 8194-row table [-inf, 0, pos0, neg0, pos1, neg1, ...]
       with pos = x (log-probs), neg = log(1 - exp(x)), per batch column.
  4 alternating layers, each: gather rows by ptrs, then segment-reduce over
  contiguous fanin groups (fanin 4 sum-of-logs "product" layers, fanin 2
  logsumexp "sum" layers).

Strategy (2-way batch x 4-way output-group sharding, 8 NeuronCores):
  - The DMA cost model charges per gather descriptor max(bytes, 512)/bus:
    sub-512B descriptors run at half rate, and the gather ISA requires
    256B-multiple rows.  64-col fp32 rows (256B) therefore cost exactly as
    much as 256-col fp16 rows (512B) PER DESCRIPTOR - so we make each
    descriptor carry 4x the batch columns at half precision and shard the
    *output groups* instead of sharding the batch further.
  - Shard the 512 batch columns 2 ways (256 per core, one fp16 table row =
    512B); within a column block, shard the 2048 output groups 4 ways.
    Each core computes only the stage-A (fused layer 0+1) groups its own
    output quarter references: ~2.6K of the 5.9K used groups (the quarters
    are chosen by sorting output groups on their min stage-A reference,
    which clusters shared references and cuts ~11% of the redundancy).
    Group computations are duplicated ~1.8x across the 4 quarters, but
    descriptor count per core drops 4x - net ~2x less DMA time, with zero
    inter-core communication.
  - Per core every table lives in DRAM as [rows, 256] fp16; one row = 512B.
  - Gathers use the SWDGE dma_gather instruction: int16 index list in SBUF,
    each index pulls one 512B row from the DRAM table.
  - DAG pruning (host side, per input set): working back from the core's
    512 output rows, only rows actually referenced downstream are computed.
  - Layer fusion: layers 0+1 fuse into stage A, layers 2+3 into stage B
    (8 gathered leaf rows per group: 2 product quads, summed on DVE, then
    logsumexp'd).
  - Cross-layer software pipelining: stage A's output rows are produced in
    chunk order, A groups are sorted by the max table-0 row they reference,
    and every gather chunk's source AP is narrowed to the exact table
    prefix it needs, so the tile framework only serializes a gather
    against the stores that overlap its prefix.  The encode is likewise
    chunked, with vars laid out partition-major per chunk (var
    1024c + 8p + s at partition p, slot s of chunk c) so each partition
    stores one contiguous 8KB run per chunk and chunk c fills the row
    prefix [2+2048c, 2+2048(c+1)).
  - All four quarters are padded to a common group count and share one
    compiled program (chunk source prefixes take the max over quarters);
    only the index lists and x differ per core.
  - Sum reduction: lse(a,b) = ln(e^a + e^b) computed directly (1 DVE add;
    Exp/Ln on the lightly-loaded ACT engine) - resolve_direct() verifies
    on the host, from the actual x values, that every e^arg stays a
    normal f32, falling back to max + ln(1+exp(min-max)) otherwise.  The
    Exp+Ln activation table is preloaded once (set 6) so the compiler
    inserts no per-instruction table reloads.
  - fp16 tables keep ample accuracy for the 2e-2 gate: host-emulated
    pipeline error is ~1.6e-3 max on the reference data.
"""

import numpy as np

P = 128
B = 256  # batch columns per core (2-way batch shard)
NCOLB = 2  # column blocks
NQ = 4  # output-group quarters per column block
NCORES = NCOLB * NQ
N_VARS = 4096
BATCH = 512
TAB0 = 2 * N_VARS + 2  # 8194
OUT_SIZES = [16384, 8192, 4096, 2048]
FANINS = [4, 2, 4, 2]
FE = 8  # edges per fused group: 2 (sum fanin) x 4 (product fanin)
GPC = 256  # max gather groups per dma_gather chunk (2048 indices)
S_ENC = N_VARS // P  # 32 encode slots per partition
ENC_CHUNKS = 8
SE = S_ENC // ENC_CHUNKS  # slots per encode chunk
VPC = P * SE  # vars per encode chunk
OUT_PER_Q = OUT_SIZES[3] // NQ  # 512 output groups per quarter


def _greedy_var_order(srcA_list):
    """Order vars so stage-A groups become gatherable progressively.

    With a fixed var order, a group is ready once the encode chunk holding
    its max-position var has stored; with 8 uniform refs per group a random
    order leaves almost every group waiting for the last chunk.  Greedy
    min-new-vars-first ordering (classic set-cover heuristic, run jointly
    over all quarters' group lists) packs co-referenced vars early so
    gathers can start after the first encode chunks and overlap the rest.
    """
    import heapq

    var_sets = []
    for srcA in srcA_list:
        for grp in srcA:
            var_sets.append(np.unique((grp[grp >= 2] - 2) // 2))
    var_insts = [[] for _ in range(N_VARS)]
    for i, vs in enumerate(var_sets):
        for v in vs:
            var_insts[v].append(i)
    remaining = [len(vs) for vs in var_sets]
    done = [False] * len(var_sets)
    placed = np.zeros(N_VARS, dtype=bool)
    heap = [(remaining[i], i) for i in range(len(var_sets))]
    heapq.heapify(heap)
    order = []
    while heap:
        r, i = heapq.heappop(heap)
        if done[i] or r != remaining[i]:
            continue
        done[i] = True
        for v in var_sets[i]:
            if not placed[v]:
                placed[v] = True
                order.append(v)
                for j in var_insts[v]:
                    if not done[j]:
                        remaining[j] -= 1
                        heapq.heappush(heap, (remaining[j], j))
    order.extend(np.nonzero(~placed)[0].tolist())
    inv = np.asarray(order, dtype=np.int64)  # position -> original var
    pi = np.empty(N_VARS, dtype=np.int64)  # original var -> position
    pi[inv] = np.arange(N_VARS)
    return pi, inv


def _pad_groups(n):
    return -(-n // P) * P


def _chunk_group_counts(n_groups, tail_ramp):
    """Groups per dma_gather chunk (FE edges per group). Sizes ramp up at
    the start (small first chunk -> quick desc-gen once the source prefix
    lands) and down at the end (short compute tail -> the last store lands
    early and the consumer unblocks sooner)."""
    g = P
    rem = n_groups
    tail = []
    for s in tail_ramp:
        if rem >= s + g:
            tail.append(s)
            rem -= s
    head = []
    for s in (128, 256):
        if rem >= s + g:
            head.append(s)
            rem -= s
    mid = []
    while rem > 0:
        s = min(GPC, rem)
        mid.append(s)
        rem -= s
    return head + mid + tail[::-1]


def plan(ptrs_list):
    """Partition output groups into quarters, prune each quarter's DAG
    bottom-up, fuse layer pairs, readiness-order stage A.

    Returns (stageA, stageB) dicts with common (padded) shape metadata and
    per-quarter index data:
      n_groups: padded group count, common across quarters
      n_src_rows: rows of the gathered table (A: TAB0, B: nA)
      chunks: list of (n_groups_in_chunk, src_prefix_rows); prefixes are
              maxed over quarters so one compiled program serves all cores
      edge_src: per-quarter list of per-edge source rows, production order
    """
    p0, p1, p2, p3 = [np.asarray(p).astype(np.int64) for p in ptrs_list]
    b_l2 = p3.reshape(-1, 2)  # [2048, 2] L2 group ids
    b_src1 = p2.reshape(-1, 4)[b_l2]  # [2048, 2, 4] L1 ids per out group
    # Quarter assignment: sort out-groups by min L1 reference so groups
    # sharing stage-A work land in the same quarter (cuts ~11% of the
    # cross-quarter duplication vs contiguous quarters).
    qorder = np.argsort(b_src1.reshape(OUT_SIZES[3], -1).min(axis=1), kind="stable")
    quarters = [qorder[OUT_PER_Q * q : OUT_PER_Q * (q + 1)] for q in range(NQ)]

    per_q = []
    for q in range(NQ):
        bq = b_src1[quarters[q]]  # [512, 2, 4]
        used1 = np.unique(bq)
        a_l0 = p1.reshape(-1, 2)[used1]  # [n1, 2] L0 group ids
        a_src0 = p0.reshape(-1, 4)[a_l0]  # [n1, 2, 4] t0 rows
        per_q.append({"used1": used1, "a_src0": a_src0, "bq": bq, "ids": quarters[q]})

    # Var placement: remap t0 rows through the greedy order (row 2+2u+k of
    # the reference table lands at 2+2*pi[u]+k; rows 0/1 fixed).
    pi, var_inv = _greedy_var_order(
        [pq["a_src0"].reshape(-1, FE) for pq in per_q]
    )
    rowmap = np.empty(TAB0, dtype=np.int64)
    rowmap[0], rowmap[1] = 0, 1
    u = np.arange(N_VARS, dtype=np.int64)
    rowmap[2 + 2 * u] = 2 + 2 * pi
    rowmap[3 + 2 * u] = 3 + 2 * pi
    for pq in per_q:
        pq["a_src0"] = rowmap[pq["a_src0"]]

    nA = max(_pad_groups(pq["used1"].size) for pq in per_q)

    for pq in per_q:
        n1 = pq["used1"].size
        srcA = np.zeros((nA, FE), dtype=np.int64)
        srcA[:n1] = pq["a_src0"].reshape(n1, FE)
        # readiness of a t0 row: encode chunk order (var v=(r-2)//2 ready
        # with encode chunk v//1024; rows 0/1 ready first)
        ready = np.maximum(srcA - 2, 0) // 2 // P
        rmax = ready.max(axis=1)
        # padding groups read only row 0: ready before any encode chunk, so
        # putting them first lets chunk 0's desc-gen+gather warm up under
        # the encode instead of idling the DMA engines.
        rmax[n1:] = -1
        order = np.argsort(rmax, kind="stable")
        pq["srcA"] = srcA[order]
        prod = np.empty(nA, dtype=np.int64)
        prod[order] = np.arange(nA)  # compact A-group id -> production row

        relabel1 = prod[np.searchsorted(pq["used1"], pq["bq"])]  # [512, 2, 4]
        srcB = relabel1.reshape(-1, FE)
        # Stage B rows need no fixed order either - the host unpermutes
        # rows after the run - so readiness-sort B too: its early chunks
        # then only need a tA prefix and can overlap stage A's tail.
        orderB = np.argsort(srcB.max(axis=1), kind="stable")
        pq["srcB"] = srcB[orderB]
        prodB = np.empty(srcB.shape[0], dtype=np.int64)
        prodB[orderB] = np.arange(srcB.shape[0])  # local out g -> produced row
        pq["out_prod"] = prodB

    def mk(srcs, n_src_rows, tail_ramp):
        n_groups = srcs[0].shape[0]
        chunks = []
        g_off = 0
        for gc in _chunk_group_counts(n_groups, tail_ramp):
            m = max(int(s[g_off : g_off + gc].max()) + 1 for s in srcs)
            chunks.append((gc, m))
            g_off += gc
        return {
            "n_groups": n_groups,
            "n_src_rows": n_src_rows,
            "chunks": chunks,
            "edge_src": [s.ravel() for s in srcs],
        }

    stages = [
        mk([pq["srcA"] for pq in per_q], TAB0, (128, 128, 128, 256)),
        mk([pq["srcB"] for pq in per_q], nA, (128, 128)),
    ]
    stages[1]["out_prod"] = [pq["out_prod"] for pq in per_q]
    stages[1]["out_ids"] = [pq["ids"] for pq in per_q]
    stages[0]["var_inv"] = var_inv  # x row order for the device table
    stages[0]["direct"] = None
    stages[1]["direct"] = None
    return stages


def resolve_direct(stages, x):
    """Exact host-side bound check for the direct-form lse (e^arg must stay
    a normal f32), evaluated over all quarters."""
    x = np.asarray(x, dtype=np.float64)[stages[0]["var_inv"]]
    pos = x
    neg = np.log1p(-np.exp(x))
    t0 = np.empty((TAB0, x.shape[1]))
    t0[0] = 0.0
    t0[1] = 0.0
    t0[2::2] = pos
    t0[3::2] = neg
    mn_a = 0.0
    mn_b = 0.0
    for q in range(NQ):
        srcA = stages[0]["edge_src"][q].reshape(-1, 2, 4)
        qv = t0[srcA].sum(axis=2)  # [nA, 2, cols]
        m = qv.max(axis=1)
        lo = qv.min(axis=1)
        tA = m + np.log1p(np.exp(lo - m))
        srcB = stages[1]["edge_src"][q].reshape(-1, 2, 4)
        tb = tA[srcB].sum(axis=2)
        mn_a = min(mn_a, qv.min())
        mn_b = min(mn_b, tb.min())
    stages[0]["direct"] = bool(mn_a > -80.0)
    stages[1]["direct"] = bool(mn_b > -80.0)


def reorder_wrap(stage, q):
    """Permute quarter q's per-edge source ids into dma_gather order and
    wrap into the int16 [128, n_edges//16] SBUF layout (position j ->
    [j%16, j//16], replicated across the 8 gpsimd cores' 16-partition
    groups).

    Edge position j of chunk ci maps to partition p=j%128, slot=j//128,
    cc=slot//FE, k=slot%FE, production row = base_ci + p*csub + cc."""
    src = stage["edge_src"][q]
    out = np.empty(stage["n_groups"] * FE, dtype=np.int64)
    base = 0
    e_off = 0
    for gc, _m in stage["chunks"]:
        csub = gc // P
        n_e = gc * FE
        j = np.arange(n_e)
        p = j % P
        slot = j // P
        cc = slot // FE
        k = slot % FE
        row = base + p * csub + cc
        out[e_off : e_off + n_e] = src[row * FE + k]
        base += gc
        e_off += n_e
    assert out.max() < 2**15 and out.min() >= 0
    out = out.astype(np.int16)
    return np.ascontiguousarray(np.tile(out.reshape(-1, 16).T, (8, 1)))


def build_nc(meta):
    """meta: per-stage (n_groups, n_src_rows, chunks-tuple, direct)."""
    import concourse.bacc as bacc
    import concourse.mybir as mybir
    import concourse.tile as tile

    f32 = mybir.dt.float32
    f16 = mybir.dt.float16
    i16 = mybir.dt.int16
    Alu = mybir.AluOpType
    Act = mybir.ActivationFunctionType

    specs = [
        {"n_groups": n, "n_src_rows": s, "chunks": ch, "direct": dr}
        for (n, s, ch, dr) in meta
    ]

    nc = bacc.Bacc("TRN2", target_bir_lowering=False, debug=False)
    x = nc.dram_tensor("x", [P, S_ENC * B], f16, kind="ExternalInput")
    idx_in = [
        nc.dram_tensor(
            f"idx{l}", [P, FE * s["n_groups"] // 16], i16, kind="ExternalInput"
        )
        for l, s in enumerate(specs)
    ]
    out = nc.dram_tensor("out", [OUT_PER_Q, B], f32, kind="ExternalOutput")

    with tile.TileContext(nc) as tc:
        with (
            tc.tile_pool(name="dram", bufs=1, space="DRAM") as dpool,
            tc.tile_pool(name="sb", bufs=4) as gp,
            tc.tile_pool(name="enc", bufs=ENC_CHUNKS) as ep,
            tc.tile_pool(name="hb", bufs=4) as hp,
            tc.tile_pool(name="tmp", bufs=3) as tp,
            tc.tile_pool(name="ix", bufs=1) as ixp,
        ):
            tables = [
                dpool.tile([s["n_src_rows"], B], f16, name=f"t{l}", tag=f"t{l}")
                for l, s in enumerate(specs)
            ]

            # Preload the combined Exp+Ln activation table once; the
            # insert_act_table_loads pass then finds every Exp/Ln already
            # covered and inserts no per-instruction reloads (1283ns each).
            ACT_SET_LN_EXP = 6  # natural_log_exp_and_others
            nc.scalar.add_instruction(
                mybir.InstLoadActFuncSet(
                    name=nc.get_next_instruction_name(),
                    ins=[],
                    outs=[],
                    act_func_set_id=ACT_SET_LN_EXP,
                )
            )

            # table0 rows 0 (-inf in the reference, never gathered) and 1
            # (zeros). Store first so the row prefix [0,2) is ready.
            z = ixp.tile([2, B], f16, tag="z")
            nc.vector.memset(z[:], 0.0)
            nc.sync.dma_start(tables[0][:][0:2, :], z[:])

            # --- encode, chunked: var v sits at partition (v%1024)//SE,
            # slot v%SE of chunk v//1024; pos row 2+2v, neg row 3+2v.
            # Chunk j covers rows [2+2048j, 2+2048(j+1)), a row prefix, so
            # stage-A gather chunks can start before the whole encode
            # finishes.  Within a chunk each partition's 2*SE rows are
            # contiguous, so the fp16 store is 128 runs of 8KB (full-rate
            # DMA; interleaved layouts would pay the sub-512B 2x penalty).
            # All independent loads are emitted before any store so the
            # in-order DMA queue never has a compute-gated store blocking a
            # ready load: x chunks first, then the index lists.
            xv = x[:].rearrange("p (s b) -> p s b", b=B)
            xls = []
            for j in range(ENC_CHUNKS):
                xl = ep.tile([P, SE, B], f16, tag="xl")
                nc.sync.dma_start(xl[:], xv[:, j * SE : (j + 1) * SE, :])
                xls.append(xl)
            ix_t = []
            for l, s in enumerate(specs):
                t = ixp.tile([P, FE * s["n_groups"] // 16], i16, tag=f"ix{l}")
                nc.sync.dma_start(t[:], idx_in[l][:])
                ix_t.append(t)
            for j in range(ENC_CHUNKS):
                xl = xls[j]
                iv = ep.tile([P, SE, 2, B], f16, tag="enc")
                et = hp.tile([P, SE, B], f32, tag="h")
                # pos copy rides the idle DVE so ACT only runs exp+ln
                nc.vector.tensor_scalar_add(iv[:][:, :, 0, :], xl[:], 0.0)
                nc.scalar.activation(et[:], xl[:], Act.Exp)
                nc.scalar.activation(
                    iv[:][:, :, 1, :], et[:], Act.Ln, scale=-1.0, bias=1.0
                )
                r0 = 2 + 2 * P * SE * j
                r1 = 2 + 2 * P * SE * (j + 1)
                # row = r0 + 2*SE*p + 2*s + k
                nc.sync.dma_start(
                    tables[0][:][r0:r1, :].rearrange("(p s k) b -> p s k b", s=SE, k=2),
                    iv[:],
                )

            # --- fused gather + product-sum + logsumexp stages ---
            for l, s in enumerate(specs):
                last = l + 1 == len(specs)
                dst_tile = out[:] if last else tables[l + 1][:]
                # intermediate tables are fp16 (halves gather descriptor
                # bytes); the final output must stay fp32
                h_dt = f32 if last else f16
                g_off = 0
                e_off = 0
                for gc, m_src in s["chunks"]:
                    csub = gc // P
                    ch = gc * FE
                    S = ch // P
                    g = gp.tile([P, S, B], f16, tag="g")
                    nc.gpsimd.dma_gather(
                        g[:],
                        tables[l][:][0:m_src, :],
                        ix_t[l][:, e_off // 16 : (e_off + ch) // 16],
                        ch,
                        ch,
                        B,
                        single_packet=False,
                    )
                    # [p, group, pair(2), fanin(4), b]
                    v = g[:].rearrange("p (c j k) b -> p c j k b", j=2, k=4)
                    s01 = tp.tile([P, csub, 2, B], f16, tag="m")
                    s23 = tp.tile([P, csub, 2, B], f16, tag="n")
                    ss = gp.tile([P, csub, 2, B], f16, tag="s")
                    nc.vector.tensor_add(s01[:], v[:, :, :, 0, :], v[:, :, :, 1, :])
                    nc.vector.tensor_add(s23[:], v[:, :, :, 2, :], v[:, :, :, 3, :])
                    nc.vector.tensor_add(ss[:], s01[:], s23[:])
                    a = ss[:][:, :, 0, :]
                    b = ss[:][:, :, 1, :]
                    h = hp.tile([P, csub, B], h_dt, tag="h")
                    if s["direct"]:
                        # lse(a,b) = ln(e^a + e^b): host verified e^min is a
                        # normal f32 (no scaling needed). 1 DVE op; the
                        # whole-tile Exp and the Ln ride the ACT engine.
                        e = tp.tile([P, csub, 2, B], f32, tag="d")
                        d = tp.tile([P, csub, B], f32, tag="sp")
                        nc.scalar.activation(e[:], ss[:], Act.Exp)
                        nc.vector.tensor_add(
                            d[:], e[:][:, :, 0, :], e[:][:, :, 1, :]
                        )
                        nc.scalar.activation(h[:], d[:], Act.Ln)
                    else:
                        # wider range: logsumexp = max + ln(1+exp(min-max))
                        m = tp.tile([P, csub, B], f32, tag="d")
                        mn = tp.tile([P, csub, B], f32, tag="sp")
                        sp = tp.tile([P, csub, B], f32, tag="sq")
                        nc.vector.tensor_tensor(m[:], a, b, op=Alu.max)
                        nc.vector.tensor_tensor(mn[:], a, b, op=Alu.min)
                        nc.vector.tensor_tensor(mn[:], mn[:], m[:], op=Alu.subtract)
                        nc.scalar.activation(mn[:], mn[:], Act.Exp)
                        nc.scalar.activation(sp[:], mn[:], Act.Ln, bias=1.0)
                        nc.vector.tensor_add(h[:], m[:], sp[:])
                    # chunk produces rows [g_off, g_off + P*csub):
                    # row = g_off + p*csub + cc
                    nc.sync.dma_start(
                        dst_tile[g_off : g_off + P * csub, :].rearrange(
                            "(p c) b -> p (c b)", p=P
                        ),
                        h[:].rearrange("p c b -> p (c b)"),
                    )
                    g_off += P * csub
                    e_off += ch
    nc.compile()
    return nc


def host_prep(x, ptrs_list, seg_list, stages=None):
    """Host-side sharding + pruning + index preprocessing -> per-core maps."""
    x = np.asarray(x, dtype=np.float32)
    for l, (n_out, f) in enumerate(zip(OUT_SIZES, FANINS)):
        seg = np.asarray(seg_list[l]).astype(np.int64)
        expected = np.repeat(np.arange(n_out, dtype=np.int64), f)
        assert np.array_equal(seg, expected), f"layer {l}: non-uniform segments"

    if stages is None:
        stages = plan(ptrs_list)
    idx_maps = [
        {f"idx{l}": reorder_wrap(s, q) for l, s in enumerate(stages)}
        for q in range(NQ)
    ]

    xvs = []
    xp = x[stages[0]["var_inv"]]  # device var order (greedy placement)
    for cb in range(NCOLB):
        xs = xp[:, cb * B : (cb + 1) * B].astype(np.float16)
        # partition p, slot j*SE+s holds var 1024j + SE*p + s
        # (partition-major within each encode chunk; see build_nc)
        xv = np.ascontiguousarray(
            xs.reshape(ENC_CHUNKS, P, SE, B).transpose(1, 0, 2, 3)
        ).reshape(P, -1)
        xvs.append(xv)
    # core i -> column block i % NCOLB, quarter i // NCOLB
    return [{"x": xvs[i % NCOLB], **idx_maps[i // NCOLB]} for i in range(NCORES)]


def _meta(stages):
    return tuple(
        (s["n_groups"], s["n_src_rows"], tuple(s["chunks"]), bool(s["direct"]))
        for s in stages
    )


_CACHE = {}


def _get_nc(meta=None):
    if meta is None:
        meta = _CACHE.get("meta")
        assert meta is not None, "call kernel() first"
    if _CACHE.get("meta") != meta:
        _CACHE["nc"] = build_nc(meta)
        _CACHE["meta"] = meta
    return _CACHE["nc"]


def kernel(x, ptrs0, seg0, ptrs1, seg1, ptrs2, seg2, ptrs3, seg3):
    from concourse.bass_utils import run_bass_kernel_spmd

    ptrs_list = [ptrs0, ptrs1, ptrs2, ptrs3]
    stages = plan(ptrs_list)
    resolve_direct(stages, x)
    nc = _get_nc(_meta(stages))
    in_maps = host_prep(x, ptrs_list, [seg0, seg1, seg2, seg3], stages)
    res = run_bass_kernel_spmd(nc, in_maps, core_ids=list(range(NCORES)))
    full = np.empty((OUT_SIZES[3], BATCH), dtype=np.float32)
    for i in range(NCORES):
        cb, q = i % NCOLB, i // NCOLB
        rows = res.results[i]["out"][stages[1]["out_prod"][q]]
        full[stages[1]["out_ids"][q], cb * B : (cb + 1) * B] = rows
    return full
